# revision 26
# baseline (speedup 1.0000x reference)
"""MultiHeadAttention (B=2, T=4096, H=6, hs=16, C=96) Bass kernel, 8 trn2 cores.

8-core SPMD layout: core c = (batch b = c//4, lane = c%4). Lanes 0..2 each own
one head PAIR (heads 2*lane, 2*lane+1); lane 3 gets all-zero weights (its
partial output is exactly 0). Every core runs the IDENTICAL program; only the
shipped data differs (which x quarter / which weight slice), so the kernel is
SPMD-clean for one shared NEFF:

  1. transpose MY x quarter locally, then AllGather X^T over the 4-core
     batch group (the collective trigger is emitted before the gpsimd
     mask/constant builds so those fill the collective's flight time).
  2. K^T, Q^T, V for MY head pair only; attention in scores-transposed
     layout (same algorithm as the 2-core version, 1/3 of the head work).
  3. Partial projection y_part f32 = O_pair @ Wp[:, pair-cols]^T + bp/4
     (each lane folds a quarter-bias, so the 4-way ReduceScatter sum lands
     + bp exactly once), written in a rank-interleaved permutation and
     summed with TWO chunked ReduceScatters: supergroups run even-first,
     RS chunk 0 fires after sg 6 and overlaps the odd supergroups'
     attention; RS chunk c hands rank r global rows [1024r + 512c, +512).
  4. cast bf16, write y [1024, 96].

Global jax arrays stay trivially assemblable: stacking per-core shards along
axis 0 makes x0 == x.reshape(8192, 96) and y == out.reshape(2, 4096, 96).

Wire bytes per call are unchanged vs the 2-core version (~1.5MB bf16 each
way + small per-core weight slices); on-device exec drops ~2x because the
attention work is split 4 ways per batch (3 head pairs + 1 zero lane).
"""

import threading

import numpy as np
import ml_dtypes

import concourse.bass as bass
import concourse.mybir as mybir
from concourse import bacc
from concourse.tile import TileContext
from concourse.masks import make_identity

F32 = mybir.dt.float32
BF16 = mybir.dt.bfloat16

B, T, C = 2, 4096, 96
H, HS = 6, 16
NSB = T // 128   # 32 s-blocks
NSG = T // 512   # 8 query supergroups
QT = T // 4      # 1024 rows per core
N_CORES = 8
GROUPS = [[0, 1, 2, 3], [4, 5, 6, 7]]


def build_nc():
    nc = bacc.Bacc("TRN2", target_bir_lowering=False, debug=False,
                   enable_asserts=False, num_devices=N_CORES)
    x0 = nc.dram_tensor("x0", [QT, C], BF16, kind="ExternalInput")
    # my pair's weights: rows 0,1 = Wq, 2,3 = Wk, 4,5 = Wv (lane 3: zeros)
    wqkv = nc.dram_tensor("wqkv", [6, C, HS], BF16, kind="ExternalInput")
    # wpb rows 0..95 = Wp[:, 32g:32g+32] (lane 3: zeros); rows 96..98 = bp/3x32
    wpb = nc.dram_tensor("wpb", [C + 3, 32], F32, kind="ExternalInput")
    y = nc.dram_tensor("y", [QT, C], BF16, kind="ExternalOutput")

    with TileContext(nc) as tc:
        with (
            tc.tile_pool(name="one", bufs=1) as one,
            tc.tile_pool(name="stg", bufs=2) as stg,
            tc.tile_pool(name="pp", bufs=4) as pp,
            tc.tile_pool(name="wk2", bufs=2) as wk2,
            tc.tile_pool(name="sps", bufs=2, space="PSUM") as sps,
            tc.tile_pool(name="ops", bufs=2, space="PSUM") as ops,
            tc.tile_pool(name="dram", bufs=1, space="DRAM") as dram,
        ):
            ident = one.tile([128, 128], F32, tag="ident")
            make_identity(nc, ident)
            identb = one.tile([128, 128], BF16, tag="identb")
            make_identity(nc, identb)

            # ---- transpose MY x quarter, AllGather X^T (trigger early so
            # the mask/constant gpsimd work below runs during the flight) ----
            xtl_d = dram.tile([C, QT], BF16, tag="xtld")
            for tb in range(QT // 128):
                xt = stg.tile([128, C], BF16, tag="xt")
                nc.sync.dma_start(out=xt, in_=x0[128 * tb:128 * (tb + 1), :])
                ps = sps.tile([C, 128], BF16, tag="S")
                nc.tensor.transpose(ps, xt, identb)
                xts = stg.tile([C, 128], BF16, tag="xts")
                nc.vector.tensor_copy(xts, ps)
                nc.sync.dma_start(out=xtl_d[:, 128 * tb:128 * (tb + 1)],
                                  in_=xts)
            xtg_d = dram.tile([4 * C, QT], BF16, tag="xtgd")
            nc.gpsimd.collective_compute(
                "AllGather", mybir.AluOpType.bypass, replica_groups=GROUPS,
                ins=[xtl_d.opt()], outs=[xtg_d.opt()])

            # diagonal-block causal masks (identical on every core)
            msk = []
            for d in range(4):
                mf = stg.tile([128, 1024], F32, tag="mstg")
                nc.gpsimd.memset(mf, 1.0)
                for l in range(2):
                    nc.gpsimd.affine_select(
                        out=mf[:, 512 * l:512 * (l + 1)],
                        in_=mf[:, 512 * l:512 * (l + 1)],
                        compare_op=mybir.AluOpType.is_ge,
                        fill=0.0, base=-128 * d,
                        pattern=[[1, 512]], channel_multiplier=-1)
                mb = one.tile([128, 1024], BF16, tag=f"msk{d}")
                nc.vector.tensor_copy(mb, mf)
                msk.append(mb)

            # normalization constants (denominator row at strip row 0).
            # bf16 throughout the normalization/projection matmuls: fp32
            # matmuls run at 4 cycles/row vs bf16's 1 on the PE, and the
            # constants are exact 0/1 while o_nrm/of/wp rounding adds well
            # under the error gate.
            Em = one.tile([64, 64], BF16, tag="Em")
            nc.gpsimd.memset(Em, 0.0)
            urow = one.tile([1, 64], BF16, tag="urow")
            nc.gpsimd.memset(urow, 0.0)
            for l in range(2):
                nc.gpsimd.memset(Em[32 * l:32 * l + 1,
                                    32 * l:32 * l + HS + 1], 1.0)
                nc.gpsimd.memset(urow[0:1, 32 * l + HS + 1:32 * l + 32], 1.0)
            ones_r = one.tile([1, 512], BF16, tag="ones")
            nc.gpsimd.memset(ones_r, 1.0)

            # padded projection weights for MY pair
            wq_pad = one.tile([C, 64], BF16, tag="wq")
            wk_pad = one.tile([C, 64], BF16, tag="wk")
            for t, off in ((wq_pad, 0), (wk_pad, 2)):
                nc.gpsimd.memset(t, 0.0)
                for l in range(2):
                    nc.sync.dma_start(out=t[:, 32 * l:32 * l + HS],
                                      in_=wqkv[off + l])
            wv_cat = one.tile([C, 2 * HS], BF16, tag="wvcat")
            for h in range(2):
                nc.sync.dma_start(out=wv_cat[:, HS * h:HS * h + HS],
                                  in_=wqkv[4 + h])
            # Wp^T padded: rows 32l+1+d <- Wp[:, 16(2g+l)+d] (= wpb col 16l+d)
            s = stg.tile([C, 64], F32, tag="wstg")
            nc.gpsimd.memset(s, 0.0)
            for l in range(2):
                nc.sync.dma_start(out=s[:, 32 * l + 1:32 * l + 1 + HS],
                                  in_=wpb[0:C, HS * l:HS * l + HS])
            psw = sps.tile([64, C], F32, tag="S")
            nc.tensor.transpose(psw, s, ident[:C, :C])
            wp_pad = one.tile([64, C], BF16, tag="wp")
            nc.vector.tensor_copy(wp_pad, psw)
            # bias broadcast [128, 96] from wpb rows 96..98
            bp_b = one.tile([128, C], F32, tag="bpb")
            for k in range(3):
                bpap = wpb[C + k]
                nc.sync.dma_start(out=bp_b[:, 32 * k:32 * k + 32], in_=bass.AP(
                    tensor=bpap.tensor, offset=bpap.offset,
                    ap=[[0, 128]] + list(bpap.ap)))

            # ---- X^T (full batch): assemble the gathered quarters ----
            xT = one.tile([C, T], BF16, tag="xT")
            for r in range(4):
                nc.sync.dma_start(out=xT[:, QT * r:QT * (r + 1)],
                                  in_=xtg_d[C * r:C * (r + 1), :])

            # ---- K^T, Q^T, V_store for MY pair ----
            kT = one.tile([64, T], BF16, tag="kT")
            qT = one.tile([64, T], BF16, tag="qT")
            for wpad, t in ((wk_pad, kT), (wq_pad, qT)):
                for cc in range(T // 512):
                    ps = sps.tile([64, 512], F32, tag="S")
                    nc.tensor.matmul(ps, wpad,
                                     xT[:, 512 * cc:512 * (cc + 1)],
                                     start=True, stop=True)
                    nc.vector.tensor_copy(t[:, 512 * cc:512 * (cc + 1)], ps)
            # vst col 0 per strip is the ones column (softmax denominator)
            vst = one.tile([128, NSB, 2, 32], BF16, tag="vst")
            nc.gpsimd.memset(vst, 0.0)
            for l in range(2):
                nc.gpsimd.memset(vst[:, :, l, 0:1], 1.0)
            for tb in range(NSB):
                ps = sps.tile([128, 2 * HS], F32, tag="S")
                nc.tensor.matmul(ps, xT[:, 128 * tb:128 * (tb + 1)], wv_cat,
                                 start=True, stop=True)
                nc.vector.tensor_copy(
                    vst[:, tb, :, 1:17],
                    ps.rearrange("p (h d) -> p h d", d=HS))

            # ---- attention + partial projection, per query supergroup ----
            # Supergroups run even-first (0,2,4,6 then 1,3,5,7) and the
            # partial projection lands in a rank-interleaved permutation of
            # ypart, so ReduceScatter can run in two chunks: chunk 0 (even
            # sgs) overlaps the odd sgs' attention, and each RS chunk c
            # hands rank r exactly global rows [1024r + 512c, +512) — its
            # own y half.
            # Partials carry bias/4 each (folded host-side into wpb rows
            # 96..98), so the bf16 ReduceScatter's 4-way sum lands the full
            # bias and the post-RS path is a single DRAM->DRAM DMA.
            ypart = dram.tile([T, C], F32, tag="ypart")
            yrs = dram.tile([QT, C], F32, tag="yrs")

            def rs_chunk(c):
                nc.gpsimd.collective_compute(
                    "ReduceScatter", mybir.AluOpType.add,
                    replica_groups=GROUPS,
                    ins=[ypart[2048 * c:2048 * (c + 1), :].opt()],
                    outs=[yrs[512 * c:512 * (c + 1), :].opt()])
                for tb in range(4):
                    r_t = stg.tile([128, C], F32, tag="rt")
                    nc.sync.dma_start(
                        out=r_t, in_=yrs[512 * c + 128 * tb:
                                         512 * c + 128 * (tb + 1), :])
                    y_sb = wk2.tile([128, C], BF16, tag="ysb2")
                    nc.vector.tensor_copy(y_sb, r_t)
                    nc.sync.dma_start(
                        out=y[512 * c + 128 * tb:512 * c + 128 * (tb + 1), :],
                        in_=y_sb)

            # Normalization + projection of supergroup k are DEFERRED into
            # supergroup k+1's s-block stream: the PE queue is in-order, so
            # emitting the proj matmuls (which wait on VectorE's reciprocal)
            # right after sg k's last PV matmul would stall the next sg's
            # S^T matmuls behind them (~2.9us per boundary).
            def norm_part(sg, o_ps):
                o_nrm = wk2.tile([64, 512], BF16, tag="onrm")
                for l in range(2):
                    nc.vector.tensor_copy(o_nrm[32 * l:32 * l + 32, :], o_ps[l])
                r_ps = sps.tile([64, 512], F32, tag="S")
                nc.tensor.matmul(r_ps, Em, o_nrm, start=True, stop=False)
                nc.tensor.matmul(r_ps, urow, ones_r, start=False, stop=True)
                r_sb = wk2.tile([64, 512], F32, tag="rsb")
                nc.vector.reciprocal(r_sb, r_ps)
                of = wk2.tile([64, 512], BF16, tag="of")
                nc.vector.tensor_mul(of, o_nrm, r_sb)
                return of

            def proj_part(sg, of):
                # permuted ypart destination: chunk (sg%2), rank slot (sg//2)
                dest = 2048 * (sg % 2) + 512 * (sg // 2)
                for st in range(4):
                    y_ps = ops.tile([128, C], F32, tag="O0")
                    nc.tensor.matmul(y_ps, of[:, 128 * st:128 * (st + 1)],
                                     wp_pad, start=True, stop=True)
                    y_sb = wk2.tile([128, C], F32, tag="ysb")
                    nc.vector.tensor_add(y_sb, y_ps, bp_b)  # bp_b holds bp/4
                    nc.sync.dma_start(
                        out=ypart[dest + 128 * st:dest + 128 * (st + 1), :],
                        in_=y_sb)
                if sg == 6:
                    rs_chunk(0)   # even sgs done: overlap RS0 with odd sgs

            pend_norm = pend_proj = None
            for sg in [0, 2, 4, 6, 1, 3, 5, 7]:
                n_sb = 4 * sg + 4
                o_ps = [ops.tile([32, 512], F32, tag=f"O{l}", name=f"ops{l}")
                        for l in range(2)]
                for sb in range(n_sb):
                    s_ps = sps.tile([128, 1024], F32, tag="S")
                    for l in range(2):
                        nc.tensor.matmul(
                            s_ps[:, 512 * l:512 * (l + 1)],
                            kT[32 * l:32 * l + HS, 128 * sb:128 * (sb + 1)],
                            qT[32 * l:32 * l + HS, 512 * sg:512 * (sg + 1)],
                            start=True, stop=True)
                    p = pp.tile([128, 1024], BF16, tag="P")
                    nc.scalar.activation(p, s_ps,
                                         mybir.ActivationFunctionType.Exp,
                                         scale=0.25)
                    d = sb - 4 * sg
                    if d >= 0:
                        nc.vector.tensor_mul(p, p, msk[d])
                    for l in range(2):
                        nc.tensor.matmul(
                            o_ps[l],
                            vst[:, sb, l, :],
                            p[:, 512 * l:512 * (l + 1)],
                            start=(sb == 0), stop=(sb == n_sb - 1))
                    if sb == 1 and pend_norm is not None:
                        pend_proj = (pend_norm[0], norm_part(*pend_norm))
                        pend_norm = None
                    if sb == 3 and pend_proj is not None:
                        proj_part(*pend_proj)
                        pend_proj = None
                pend_norm = (sg, o_ps)
            pend_proj = (pend_norm[0], norm_part(*pend_norm))
            proj_part(*pend_proj)
            rs_chunk(1)
    nc.finalize()
    return nc


_NC_CACHE = {}
_NC_LOCK = threading.Lock()


def _fast_runner(nc):
    """Persistent shard_map jit over 8 cores (reusable across calls)."""
    import jax
    from jax.sharding import Mesh, PartitionSpec
    from jax.experimental.shard_map import shard_map
    from concourse import bass2jax
    bass2jax.install_neuronx_cc_hook()
    in_names, out_names, out_avals = [], [], []
    in_specs_sd = []
    for alloc in nc.m.functions[0].allocations:
        if not isinstance(alloc, mybir.MemoryLocationSet):
            continue
        name = alloc.memorylocations[0].name
        if alloc.kind == "ExternalInput":
            if nc.partition_id_tensor is None or name != nc.partition_id_tensor.name:
                in_names.append(name)
                in_specs_sd.append((tuple(alloc.tensor_shape),
                                    mybir.dt.np(alloc.dtype)))
        elif alloc.kind == "ExternalOutput":
            out_names.append(name)
            shape = tuple(alloc.tensor_shape)
            dtype = mybir.dt.np(alloc.dtype)
            out_avals.append(jax.core.ShapedArray(shape, dtype))
    n_params = len(in_names)
    all_names = list(in_names)
    if nc.partition_id_tensor is not None:
        all_names = all_names + [nc.partition_id_tensor.name]

    def _body(*args):
        ops_ = list(args)
        if nc.partition_id_tensor is not None:
            ops_.append(bass2jax.partition_id_tensor())
        return tuple(bass2jax._bass_exec_p.bind(
            *ops_, out_avals=tuple(out_avals), in_names=tuple(all_names),
            out_names=tuple(out_names), lowering_input_output_aliases=(),
            sim_require_finite=True, sim_require_nnan=True, nc=nc))

    devices = jax.devices()[:N_CORES]
    mesh = Mesh(np.asarray(devices), ("core",))

    donate = (in_names.index("x0"),) if "x0" in in_names else ()

    def make_jit():
        return jax.jit(shard_map(_body, mesh=mesh,
                                 in_specs=(PartitionSpec("core",),) * n_params,
                                 out_specs=(PartitionSpec("core"),) * len(out_names),
                                 check_rep=False), keep_unused=True,
                       donate_argnums=donate)

    sh = jax.sharding.NamedSharding(mesh, PartitionSpec("core"))
    try:
        example = [jax.ShapeDtypeStruct((N_CORES * s[0], *s[1:]), dt, sharding=sh)
                   for s, dt in in_specs_sd]
        sharded = bass2jax.fast_dispatch_compile(
            lambda: make_jit().lower(*example).compile())
    except Exception:
        sharded = make_jit()

    yidx = out_names.index("y")

    class Runner:
        def run(self, in_map):
            outs = sharded(*[in_map[nm] for nm in in_names])
            return np.asarray(outs[yidx])

    return Runner()


def _per_core_weights(Wq, Wk, Wv, Wp, bp):
    """Per-lane weight slices; lane 3 gets zeros (its partial y is 0)."""
    BF = ml_dtypes.bfloat16
    wqkv_l, wpb_l = [], []
    for lane in range(4):
        if lane < 3:
            qkv = np.concatenate([Wq[2 * lane:2 * lane + 2],
                                  Wk[2 * lane:2 * lane + 2],
                                  Wv[2 * lane:2 * lane + 2]], axis=0)
            wp_sl = Wp[:, 32 * lane:32 * lane + 32].astype(np.float32)
        else:
            qkv = np.zeros((6, C, HS), np.float32)
            wp_sl = np.zeros((C, 32), np.float32)
        wqkv_l.append(qkv.astype(BF))
        # bp/4: each of the 4 lanes folds a quarter-bias into its partial,
        # so the ReduceScatter's 4-way sum yields + bp exactly once.
        wpb_l.append(np.concatenate([wp_sl, (bp / 4.0).reshape(3, 32)],
                                    axis=0).astype(np.float32))
    return wqkv_l, wpb_l


def kernel(x, Wq, Wk, Wv, Wp, bp):
    x = np.asarray(x, np.float32)
    with _NC_LOCK:
        if "nc" not in _NC_CACHE:
            _NC_CACHE["nc"] = build_nc()
    nc = _NC_CACHE["nc"]

    BF = ml_dtypes.bfloat16
    Wq, Wk, Wv = (np.asarray(w, np.float32) for w in (Wq, Wk, Wv))
    Wp = np.asarray(Wp, np.float32)
    bp = np.asarray(bp, np.float32)
    wqkv_l, wpb_l = _per_core_weights(Wq, Wk, Wv, Wp, bp)

    xbf = np.ascontiguousarray(x).astype(BF)        # [B, T, C]

    try:
        if "runner" not in _NC_CACHE:
            _NC_CACHE["runner"] = _fast_runner(nc)
        in_map = {
            # core c = (batch c//4, lane c%4); lane r owns rows [1024r, +1024)
            # so the global stack is just x.reshape(B*T, C)
            "x0": xbf.reshape(B * T, C),
            "wqkv": np.concatenate(wqkv_l * B, axis=0),
            "wpb": np.concatenate(wpb_l * B, axis=0),
        }
        yflat = _NC_CACHE["runner"].run(in_map)
    except Exception:
        from concourse import bass_utils
        in_maps = [{"x0": xbf[c // 4, QT * (c % 4):QT * (c % 4 + 1)],
                    "wqkv": wqkv_l[c % 4], "wpb": wpb_l[c % 4]}
                   for c in range(N_CORES)]
        results = bass_utils.run_bass_kernel_spmd(
            nc, in_maps, core_ids=list(range(N_CORES))).results
        yflat = np.concatenate([results[c]["y"] for c in range(N_CORES)],
                               axis=0)
    return np.asarray(yflat).astype(np.float32).reshape(B, T, C)
